# revision 63
# baseline (speedup 1.0000x reference)
"""Trainium2 Bass kernel for nn_AttentionHead_28389733827022.

Reference (faithful to source, including the v=q bug):
    q = x @ Wq + bq; k = x @ Wk + bk; v = q
    scores = einsum("bqd,bkd->bqk", q, k) / sqrt(S)
    attn   = softmax(scores, axis=1)          # over the QUERY axis
    out    = einsum("bqk,bkd->bqd", attn, v)

B=8 batches -> one batch element per NeuronCore (pure data parallel, no
collectives). Per-core modeled time ~58us; ACT (exp) is the bottleneck
engine (~43us busy) and the schedule keeps it >95% utilized.

Layout strategy (matmuls contract over the partition dim):
  - host supplies xT = x.T [E,S] in bf16 so projections contract E, plus
    packed W = [Wq|Wk] [E,2D] and b = [bq;bk]: ONE projection pass gives
    qkT [2D,S] = [qT;kT] stacked on partitions
  - scores_T[k,q] = kT_tile^T @ qT (K=D=64): the softmax axis (q) becomes
    the FREE axis, so exp runs on ACT with the 1/sqrt(S) scale fused and
    accum_out producing the per-k row sums for free
  - softmax normalizer folds into v rows (v_scaled[k,:] = v[k,:]/sum), so
    no 4M-element normalization pass exists
  - v = q in [S,D] layout via 16 PE transposes of qT tiles
  - out^T[d,q] += v_scaled_tile^T @ expT_tile accumulates in PSUM over the
    16 k-tiles (two half-width psum tensors so the two tail evacuation
    copies overlap); out ships bf16, host transposes/upcasts

Schedule (single TileContext; per-engine order = emission order):
  - input DMAs: x chunk loads on the SP HWDGE ring, small/lenient loads on
    the ACT ring (each dma_start costs ~0.6us serialized ring issue)
  - ~64 dummy ident matmuls warm the PE HAM clock gate during the DMA wait
  - projections run in 256/512-wide q pieces; the first exp fires at
    ~7.4us, right after the first 256 columns are projected; kT pieces
    that gate early exps are recomputed at base partition 0 (matmul lhsT
    and rhs must share base_partition) instead of waiting for SBUF DMAs
  - loop1 emits h0 scores/exp for tiles 0..3 in readiness order to bridge
    ACT until all projection chunks land
  - merged loop: per k-tile h0+h1 exp, sums->reciprocal->v_scale on DVE,
    AV matmuls; AV/transpose work drains from a backlog a few ops at a
    time so the PE stream never inserts a long burst between the score
    matmuls that feed ACT (ACT 2.46us/tile vs PE ~2.3us/tile)

_build(n_iter>1) chains serialized copies of the whole kernel in one NEFF
(poison DMA ties iteration i+1's input load to iteration i's output) for
wall-clock timing experiments; the deliverable path uses n_iter=1.
"""

import sys

if "/opt/trn_rl_repo" not in sys.path:
    sys.path.insert(0, "/opt/trn_rl_repo")

from contextlib import ExitStack
from math import sqrt

import numpy as np
import ml_dtypes

import concourse.bass as bass
import concourse.tile as tile
from concourse import bacc, mybir
from concourse.bass_utils import run_bass_kernel_spmd
from concourse.masks import make_identity

B, S, E, D = 8, 2048, 768, 64
P = 128
ET = E // P          # 6 e-tiles for the E contraction
KT = S // P          # 16 k-tiles over the key/sequence axis
CH = 512             # matmul moving-dim chunk (one PSUM bank of f32)
NCH = S // CH        # 4 chunks of the q axis
SCALE = 1.0 / sqrt(S)

BF16 = mybir.dt.bfloat16
F32 = mybir.dt.float32
ts = bass.ts
Exp = mybir.ActivationFunctionType.Exp


def _build(n_iter=1):
    nc = bacc.Bacc("TRN2", target_bir_lowering=False, debug=False, num_devices=B)

    xT = nc.dram_tensor("xT", [E, S], BF16, kind="ExternalInput").ap()
    # w arrives pre-arranged partition-major ([P, ET*2D]) so the DMA moves
    # one contiguous 1.5KB run per partition instead of 6x256B pieces
    w = nc.dram_tensor("w", [P, ET * 2 * D], BF16, kind="ExternalInput").ap()
    b = nc.dram_tensor("b", [2 * D, 1], F32, kind="ExternalInput").ap()
    out = nc.dram_tensor("out", [D, S], BF16, kind="ExternalOutput").ap()

    with tile.TileContext(nc) as tc:
        for it in range(n_iter):
            _emit_iter(nc, tc, xT, w, b, out, poison=(it > 0))

    nc.compile()
    return nc


def _emit_iter(nc, tc, xT, w, b, out, poison=False):
    xT_t = xT.rearrange("(t p) s -> p t s", p=P)

    with ExitStack() as ctx:
        const = ctx.enter_context(tc.tile_pool(name="const", bufs=1))
        big = ctx.enter_context(tc.tile_pool(name="big", bufs=1))
        work_sb = ctx.enter_context(tc.tile_pool(name="work_sb", bufs=2))

        xT_sb = big.tile([P, ET, S], BF16, tag="xT")
        w_sb = const.tile([P, ET, 2 * D], BF16, tag="w")
        w_t = w.rearrange("p (t d) -> p t d", t=ET)
        if poison:
            # timing builds only: serialize this iteration's input load
            # behind the previous iteration's final output write
            nc.sync.dma_start(out=xT_sb[0:1, 0, 0:64], in_=out[0:1, 0:64])
        # SP ring carries only the big loads (each dma_start costs ~0.6us of
        # serialized HWDGE issue). The first 256 columns of x land first so
        # the first projection piece starts as early as possible; tiny bias
        # loads ride the ACT ring.
        nc.sync.dma_start(out=xT_sb[:, :, 0:256], in_=xT_t[:, :, 0:256])
        nc.sync.dma_start(out=w_sb, in_=w)
        nc.sync.dma_start(out=xT_sb[:, :, 256:512], in_=xT_t[:, :, 256:512])
        b_sb = const.tile([2 * D, 1], F32, tag="b")
        nc.scalar.dma_start(out=b_sb, in_=b)
        bk_sb = const.tile([D, 1], F32, tag="bk")
        nc.scalar.dma_start(out=bk_sb, in_=b[D : 2 * D, :])
        for c in range(1, NCH):
            nc.sync.dma_start(out=xT_sb[:, :, ts(c, CH)], in_=xT_t[:, :, ts(c, CH)])
        ident = const.tile([D, D], BF16, tag="ident")
        make_identity(nc, ident)
        # dummy exp to hoist the ACT table load off the critical path
        dummy = const.tile([1, 1], F32, tag="dummy")
        nc.vector.memset(dummy, 0.0)
        nc.scalar.activation(dummy, dummy, Exp)
        qkT_sb = big.tile([2 * D, S], BF16, tag="qkT")
        v_sb = big.tile([P, KT, D], BF16, tag="v")
        qT_sb = qkT_sb[0:D, :]
        # kT must sit at base partition 0 to be a matmul lhsT alongside qT;
        # SBUF->SBUF DMA moves it down (engines can't cross partitions)
        kT_sb = big.tile([D, S], BF16, tag="kT")

        # work psum pool first so it owns low banks; proj + outT share the rest
        work_ps = ctx.enter_context(tc.tile_pool(name="work_ps", bufs=2, space="PSUM"))

        # ---- chunked projections qkT = [Wq|Wk]^T @ xT + [bq;bk] ----
        # Emission order = per-engine static program order, so the first two
        # chunks are emitted before the h0 scores loop (unblocking exp as
        # early as possible) and the last two chunks + v transposes are
        # interleaved after the first scores tile.
        def proj_piece(proj_ps, q0, qw, with_k0):
            qk_ps = proj_ps.tile([2 * D, qw], F32, tag="proj", name=f"qk_ps_{q0}")
            for e in range(ET):
                nc.tensor.matmul(
                    qk_ps,
                    w_sb[:, e, :],
                    xT_sb[:, e, q0 : q0 + qw],
                    start=(e == 0),
                    stop=(e == ET - 1),
                )
            nc.vector.tensor_scalar_add(qkT_sb[:, q0 : q0 + qw], qk_ps, b_sb)
            if with_k0 is None:
                # later pieces have lenient deadlines; copy on the ACT HWDGE
                # ring so they never block xT chunk loads on the SP ring
                nc.scalar.dma_start(
                    out=kT_sb[:, q0 : q0 + qw], in_=qkT_sb[D : 2 * D, q0 : q0 + qw]
                )

        def k0_piece(proj_ps, q0, qw):
            # early kT pieces gate the first exps: recompute at base
            # partition 0 with extra matmuls instead of waiting for a DMA
            # slot behind the xT streams
            k0_ps = proj_ps.tile([D, qw], F32, tag="proj", name=f"k0_ps_{q0}")
            for e in range(ET):
                nc.tensor.matmul(
                    k0_ps,
                    w_sb[:, e, D : 2 * D],
                    xT_sb[:, e, q0 : q0 + qw],
                    start=(e == 0),
                    stop=(e == ET - 1),
                )
            nc.vector.tensor_scalar_add(kT_sb[:, q0 : q0 + qw], k0_ps, bk_sb)

        expT = {}
        acc = {}

        def scores_piece(t, q0, qw, slot):
            sc_ps = work_ps.tile([P, 1024], F32, tag="w", name=f"sc_{t}_{q0}")
            o = 0
            while o < qw:
                w_ = min(CH, qw - o)
                nc.tensor.matmul(
                    sc_ps[:, o : o + w_],
                    kT_sb[:, ts(t, P)],
                    qT_sb[:, q0 + o : q0 + o + w_],
                    start=True,
                    stop=True,
                )
                o += w_
            nc.scalar.activation(
                expT[t][:, q0 : q0 + qw],
                sc_ps[:, 0:qw],
                Exp,
                scale=SCALE,
                accum_out=acc[t][:, slot : slot + 1],
            )

        def v_transpose(tt):
            v_ps = work_ps.tile([P, D], BF16, tag="w", name=f"v_ps_{tt}")
            nc.tensor.transpose(v_ps, qT_sb[:, ts(tt, P)], ident)
            nc.vector.tensor_copy(out=v_sb[:, tt, :], in_=v_ps)

        with tc.tile_pool(name="proj_ps", bufs=2, space="PSUM") as proj_ps:
            # warm the PE HAM clock gate during the input DMA: dummy matmuls
            # keep PE busy so the projections run at 2.4 GHz
            warm_ps = proj_ps.tile([D, D], F32, tag="warm", bufs=1)
            for i in range(64):
                nc.tensor.matmul(warm_ps, ident, ident, start=True, stop=True)

            G = 4
            NBUF = 8

            def alloc_tile(t):
                expT[t] = work_sb.tile([P, S], BF16, tag="expT", bufs=NBUF,
                                       name=f"expT_{t}")
                acc[t] = work_sb.tile([P, 6], F32, tag="acc", bufs=NBUF,
                                      name=f"acc_{t}")

            # 256-wide first pieces: the first exp fires as soon as the first
            # 256 q columns are projected, ~4us before a 512-chunk pipeline
            proj_piece(proj_ps, 0, 256, True)
            k0_piece(proj_ps, 0, 256)
            alloc_tile(0)
            scores_piece(0, 0, 256, 0)
            proj_piece(proj_ps, 256, 256, True)
            scores_piece(0, 256, 256, 1)
            proj_piece(proj_ps, 512, 256, True)
            scores_piece(0, 512, 256, 2)
            proj_piece(proj_ps, 768, 256, True)
            scores_piece(0, 768, 256, 3)
            # kT cols 256-512 (needed by tiles 2-3) deferred off the t=0 path
            k0_piece(proj_ps, 256, 256)

            # h0 for tiles 1..3 bridges ACT until the last projection chunks
            # land; late proj chunks + early v transposes interleave
            for t in range(1, G):
                alloc_tile(t)
                scores_piece(t, 0, 1024, 0)
                if t == 1:
                    proj_piece(proj_ps, 2 * CH, CH, None)
                elif t == 2:
                    proj_piece(proj_ps, 3 * CH, CH, None)
            for tt in range(G):
                v_transpose(tt)

        with tc.tile_pool(name="out_ps", bufs=1, space="PSUM") as out_ps_pool:
            # two separate psum tensors (2 banks each) so the two tail
            # copies have independent reader deps and overlap
            outT_a = out_ps_pool.tile([D, 1024], F32, tag="oa", name="outT_a")
            outT_b = out_ps_pool.tile([D, 1024], F32, tag="ob", name="outT_b")
            # merged loop: per tile, remaining exp halves + normalizer + AV.
            # ACT is the bottleneck (2.46us/tile vs ~1.9-2.3us PE). AV matmuls
            # and v transposes go through a small backlog drained a few ops at
            # a time after each scores pair, so the PE stream never inserts a
            # long burst between the matmuls that feed ACT.
            from collections import deque

            backlog = deque()
            vsc_d = {}

            def av_mm(t, c):
                dst = outT_a if c < 2 else outT_b
                nc.tensor.matmul(
                    dst[:, ts(c % 2, CH)],
                    vsc_d[t],
                    expT[t][:, ts(c, CH)],
                    start=(t == 0),
                    stop=(t == KT - 1),
                )

            def drain(n):
                for _ in range(min(n, len(backlog))):
                    backlog.popleft()()

            for t in range(KT):
                if t >= G:
                    alloc_tile(t)
                    scores_piece(t, 0, 1024, 0)
                    drain(3)
                if t == 0:
                    # first tile's h1 in two 512 pieces (projections for the
                    # last q chunks may still be landing)
                    scores_piece(0, 1024, CH, 4)
                    scores_piece(0, 1536, CH, 5)
                else:
                    scores_piece(t, 1024, 1024, 1)
                if t + G < KT:
                    backlog.append(lambda tt=t + G: v_transpose(tt))
                sums = work_sb.tile([P, 1], F32, tag="sums", name=f"sums_{t}")
                if t == 0:
                    nc.vector.tensor_add(
                        out=acc[t][:, 0:1], in0=acc[t][:, 0:1], in1=acc[t][:, 1:2]
                    )
                    nc.vector.tensor_add(
                        out=acc[t][:, 2:3], in0=acc[t][:, 2:3], in1=acc[t][:, 3:4]
                    )
                    nc.vector.tensor_add(
                        out=acc[t][:, 4:5], in0=acc[t][:, 4:5], in1=acc[t][:, 5:6]
                    )
                    nc.vector.tensor_add(
                        out=acc[t][:, 0:1], in0=acc[t][:, 0:1], in1=acc[t][:, 2:3]
                    )
                    nc.vector.tensor_add(
                        out=sums, in0=acc[t][:, 0:1], in1=acc[t][:, 4:5]
                    )
                else:
                    nc.vector.tensor_add(
                        out=sums, in0=acc[t][:, 0:1], in1=acc[t][:, 1:2]
                    )
                r = work_sb.tile([P, 1], F32, tag="r", name=f"r_{t}")
                nc.vector.reciprocal(r, sums)
                vsc = work_sb.tile([P, D], BF16, tag="vsc", bufs=4,
                                   name=f"vsc_{t}")
                nc.vector.tensor_scalar_mul(vsc, v_sb[:, t, :], r)
                vsc_d[t] = vsc
                for c in range(NCH):
                    backlog.append(lambda tt=t, cc=c: av_mm(tt, cc))
                drain(3)
            drain(len(backlog))
            # tail: evacuate out^T in two halves on independent engine+ring
            # pairs (DVE+SP ring, ACT+ACT ring) so they fully overlap
            for c in range(2):
                sl = slice(c * 1024, (c + 1) * 1024)
                o_sb = work_sb.tile([D, 1024], BF16, tag="o_sb", bufs=2,
                                    name=f"o_sb_{c}")
                if c == 0:
                    nc.vector.tensor_copy(out=o_sb, in_=outT_a)
                    nc.sync.dma_start(out=out[:, sl], in_=o_sb)
                else:
                    nc.scalar.copy(out=o_sb, in_=outT_b)
                    nc.scalar.dma_start(out=out[:, sl], in_=o_sb)


_NC_CACHE = None


def _get_nc():
    global _NC_CACHE
    if _NC_CACHE is None:
        _NC_CACHE = _build()
    return _NC_CACHE


def _in_maps(input_ids, Wq, bq, Wk, bk):
    x = np.asarray(input_ids, dtype=np.float32)
    w = np.concatenate(
        [np.asarray(Wq, np.float32), np.asarray(Wk, np.float32)], axis=1
    ).astype(ml_dtypes.bfloat16)
    # partition-major pre-arrangement: w_pre[p, e*2D+d] = w[e*P+p, d]
    w = np.ascontiguousarray(
        w.reshape(ET, P, 2 * D).transpose(1, 0, 2).reshape(P, ET * 2 * D)
    )
    bvec = np.concatenate(
        [np.asarray(bq, np.float32), np.asarray(bk, np.float32)]
    ).reshape(2 * D, 1)
    maps = []
    for i in range(B):
        xT_i = np.ascontiguousarray(x[i].T).astype(ml_dtypes.bfloat16)
        maps.append({"xT": xT_i, "w": w, "b": bvec})
    return maps


def kernel(input_ids, Wq, bq, Wk, bk, Wv, bv, **_unused):
    nc = _get_nc()
    maps = _in_maps(input_ids, Wq, bq, Wk, bk)
    res = run_bass_kernel_spmd(nc, maps, core_ids=list(range(B)))
    out = np.stack([np.asarray(res.results[i]["out"]).T for i in range(B)])
    return out.astype(np.float32)


if __name__ == "__main__":
    rng = np.random.default_rng(0)
    inputs = {
        "input_ids": rng.normal(size=(B, S, E)).astype(np.float32),
        "Wq": (rng.normal(size=(E, D)) * 0.02).astype(np.float32),
        "bq": (rng.normal(size=(D,)) * 0.02).astype(np.float32),
        "Wk": (rng.normal(size=(E, D)) * 0.02).astype(np.float32),
        "bk": (rng.normal(size=(D,)) * 0.02).astype(np.float32),
        "Wv": (rng.normal(size=(E, D)) * 0.02).astype(np.float32),
        "bv": (rng.normal(size=(D,)) * 0.02).astype(np.float32),
    }
    out = kernel(**inputs)
    print("kernel output", out.shape, out.dtype)
